# revision 1
# baseline (speedup 1.0000x reference)
"""Trainium2 Bass kernel for nn_Canny_61100204753382 (8-core SPMD).

Sharding: spatial row-bands (64 output rows x all 8 images per core). The
reference's flat-gather quirk reads all_filtered[k_pos, b, i, j] - the
direction index lands in the batch slot and the pixel's own batch index
selects the direction offset - so the coupling between images is at the SAME
pixel position and row-band sharding stays core-local given a small row halo.

Per-core device pipeline:
  stepA (PE): per-channel row-direction 11-tap composite convs
    (gauss (*) sobel-row-part) as banded matmuls -> [col, row'] layout.
  stepB (PE): col-direction 11-tap composite convs as banded matmuls -> per-
    channel gx, gy (squared via fused ACT evacuation), plus channel-summed
    gxs, gys from pre-summed stepA outputs.
  mag (ACT/DVE): per-channel sqrt(gx^2+gy^2) summed over channels -> G.
  NMS: sector class of (gxs,gys) via slope comparisons (no atan2; reproduces
    the atan2+round chain exactly); C_b = (G > shift_b(G)) for the 8 direction
    offsets (column shifts materialized by SBUF->SBUF DMA, row shifts are
    free-dim AP offsets); F_{b,j} = C_b[j] & C_b[j+4]; 4-way predicated select
    by sector class -> is_max.
  Hysteresis: out = hi | (mid & (sum3x3(hi) > hi)) - exact restructuring of
    the reference's threshold/connect logic.
Host: pads & shards input rows, assembles output bands, zeroes borders.
"""

import math
import numpy as np
from contextlib import ExitStack

import concourse.bass as bass
import concourse.mybir as mybir
import concourse.tile as tile
from concourse.bass_utils import run_bass_kernel_spmd
from concourse.alu_op_type import AluOpType

f32 = mybir.dt.float32
f32r = mybir.dt.float32r
bf16 = mybir.dt.bfloat16
u8 = mybir.dt.uint8
AF = mybir.ActivationFunctionType

B, C, H, W = 8, 3, 512, 512
NCORES = 8
RB = H // NCORES          # output rows per core
XR = RB + 14              # input rows per core (7-row halo each side)
XC = W + 14               # padded cols
GR = RB + 4               # G rows per band (final rows -2..65)
NW = 5                    # column chunks
CW = 118                  # chunk stride (128 in-cols -> 118 out-cols)
WIN = RB + 2              # is_max row window (final rows -1..64)
T1 = float(math.tan(math.pi / 8))
T2 = float(math.tan(3 * math.pi / 8))
LOW, HIGH = 0.1, 0.3
NEIGH = [(0, 1), (1, 1), (1, 0), (1, -1), (0, -1), (-1, -1), (-1, 0), (-1, 1)]

DT_CONV = f32             # f32 = exact (4 cyc/row); f32r = fast (~3e-4 err)

_CACHE = {}
TRACE = False
LAST_EXEC_NS = None


def _band(comp, K, M, taps=11):
    Wb = np.zeros((K, M), np.float32)
    for k in range(K):
        for m in range(M):
            if 0 <= k - m < taps:
                Wb[k, m] = comp[k - m]
    return Wb


def _chunk_dims(w):
    s = CW * w
    kw = min(128, XC - s)           # in-cols this chunk
    mw = min(CW, (W + 4) - s)       # out (G) cols this chunk
    return s, kw, mw


DEBUG_OUT = False


def _build():
    nc = bass.Bass()
    x_d = nc.dram_tensor("x", [XR, B * C, XC], DT_CONV, kind="ExternalInput")
    wa_d = nc.dram_tensor("wa", [XR, 2, 68], DT_CONV, kind="ExternalInput")
    wb_d = nc.dram_tensor("wb", [128, 2, 118], DT_CONV, kind="ExternalInput")
    o_d = nc.dram_tensor("o", [118, NW, B, RB], f32, kind="ExternalOutput")

    with tile.TileContext(nc) as tc, ExitStack() as ctx:
        P = ctx.enter_context
        const = P(tc.tile_pool(name="const", bufs=1))
        big = P(tc.tile_pool(name="big", bufs=1))
        ev = P(tc.tile_pool(name="ev", bufs=2))
        xp5 = P(tc.tile_pool(name="xp5", bufs=5))
        psA = P(tc.tile_pool(name="psA", bufs=2, space="PSUM"))
        psB = P(tc.tile_pool(name="psB", bufs=4, space="PSUM"))
        psS = P(tc.tile_pool(name="psS", bufs=2, space="PSUM"))

        x_sb = big.tile([XR, B * C, XC], DT_CONV, name="x_sb", tag="x_sb")
        nc.sync.dma_start(x_sb[:], x_d[:])
        wa_sb = const.tile([XR, 2, 68], DT_CONV, tag="wa_sb")
        nc.sync.dma_start(wa_sb[:], wa_d[:])
        wb_sb = const.tile([128, 2, 118], DT_CONV, tag="wb_sb")
        nc.sync.dma_start(wb_sb[:], wb_d[:])
        G = big.tile([128, NW, B, GR], f32, name="G", tag="G")
        sectors = {}
        hyst = {}

        # ---- phase 1: convs, magnitude, sector masks ----
        for w in range(NW):
            s, kw, mw = _chunk_dims(w)
            gxA = ev.tile([128, B, C, 2, 68], DT_CONV, tag="gxA")
            for img in range(B):
                pa = psA.tile([128, 3, 2, 68], f32, tag="pa")
                for ci in range(3):
                    lhsT = x_sb[0:XR, img * C + ci, s:s + kw]
                    nc.tensor.matmul(pa[0:kw, ci], lhsT, wa_sb[0:XR],
                                     start=True, stop=True)
                if img % 2 == 0:
                    nc.vector.tensor_copy(gxA[0:kw, img], pa[0:kw])
                else:
                    nc.scalar.copy(gxA[0:kw, img], pa[0:kw])
            gsA = ev.tile([128, B, 2, 68], DT_CONV, tag="gsA")
            nc.gpsimd.tensor_tensor(gsA[:], gxA[:, :, 0], gxA[:, :, 1],
                                    AluOpType.add)
            nc.gpsimd.tensor_tensor(gsA[:], gsA[:], gxA[:, :, 2], AluOpType.add)
            sq = ev.tile([128, B, 2, 3, GR], f32, tag="sq", bufs=1)
            for img in range(B):
                pb = psB.tile([118, 2, 3, 68], f32, tag="pb")
                for j in range(2):
                    nc.tensor.matmul(pb[0:mw, j], wb_sb[0:kw, j, 0:mw],
                                     gxA[0:kw, img, :, j], start=True, stop=True)
                nc.scalar.square(sq[0:mw, img], pb[0:mw])
            mag = ev.tile([128, B, 3, GR], f32, tag="mag", bufs=1)
            nc.vector.tensor_tensor(mag[0:118], sq[0:118, :, 0],
                                    sq[0:118, :, 1], AluOpType.add)
            nc.scalar.sqrt(mag[0:118], mag[0:118])
            tg = ev.tile([128, B, GR], f32, tag="tg")
            nc.vector.tensor_tensor(tg[0:118], mag[0:118, :, 0],
                                    mag[0:118, :, 1], AluOpType.add)
            nc.vector.tensor_tensor(G[0:118, w], tg[0:118],
                                    mag[0:118, :, 2], AluOpType.add)
            gxs = ev.tile([128, B, GR], f32, tag="gxs")
            gys = ev.tile([128, B, GR], f32, tag="gys")
            for j in range(2):
                for h in range(2):
                    pS = psS.tile([118, 4, 68], f32, tag="pS")
                    nc.tensor.matmul(pS[0:mw], wb_sb[0:kw, j, 0:mw],
                                     gsA[0:kw, h * 4:h * 4 + 4, j],
                                     start=True, stop=True)
                    dst = (gxs if j == 0 else gys)
                    nc.vector.tensor_copy(dst[0:118, h * 4:h * 4 + 4], pS[0:118])
            c1m = xp5.tile([128, B, WIN], u8, tag="c1m")
            c2m = xp5.tile([128, B, WIN], u8, tag="c2m")
            qsm = xp5.tile([128, B, WIN], u8, tag="qsm")
            qpr = ev.tile([128, B, WIN], f32, tag="tg")
            nc.gpsimd.tensor_tensor(qpr[0:118], gxs[0:118, :, 1:1 + WIN],
                                    gys[0:118, :, 1:1 + WIN], AluOpType.mult)
            nc.vector.tensor_single_scalar(qsm[0:118], qpr[0:118], 0.0,
                                           AluOpType.is_ge)
            nc.scalar.activation(gxs[0:118], gxs[0:118], AF.Abs)
            nc.scalar.activation(gys[0:118], gys[0:118], AF.Abs)
            wax = gxs[0:118, :, 1:1 + WIN]
            way = gys[0:118, :, 1:1 + WIN]
            nc.vector.scalar_tensor_tensor(c1m[0:118], wax, T1, way,
                                           AluOpType.mult, AluOpType.is_gt)
            nc.vector.scalar_tensor_tensor(c2m[0:118], wax, T2, way,
                                           AluOpType.mult, AluOpType.is_lt)
            sectors[w] = (c1m, c2m, qsm)

        # ---- phase 2: NMS + hysteresis rowsums per chunk ----
        for w in range(NW):
            c1m, c2m, qsm = sectors[w]

            def wsl(t, dr=0, _w=w):
                return t[0:118, _w, :, 1 + dr:1 + dr + WIN]

            Gp1 = ev.tile([128, B, GR], f32, tag="Gp1")
            Gm1 = ev.tile([128, B, GR], f32, tag="Gm1")
            nc.sync.dma_start(Gp1[0:117], G[1:118, w])
            if w + 1 < NW:
                nc.sync.dma_start(Gp1[117:118], G[0:1, w + 1])
            nc.sync.dma_start(Gm1[1:118], G[0:117, w])
            if w > 0:
                nc.sync.dma_start(Gm1[0:1], G[117:118, w - 1])
            F_all = ev.tile([128, 4, B, WIN], bf16, tag="F_all")
            for b in range(8):
                dr, dc = NEIGH[b]
                cb = ev.tile([128, B, WIN], bf16, tag="cb", bufs=3)
                shs = (wsl(G, dr) if dc == 0 else
                       {1: Gp1, -1: Gm1}[dc][0:118, :, 1 + dr:1 + dr + WIN])
                nc.vector.tensor_tensor(cb[0:118], wsl(G), shs, AluOpType.is_gt)
                nc.gpsimd.tensor_tensor(F_all[0:118, :, b], cb[0:118, 0:4],
                                        cb[0:118, 4:8], AluOpType.mult)
            sel = ev.tile([128, B, WIN], bf16, tag="sel")
            nc.vector.tensor_copy(sel[0:118], F_all[0:118, 3])
            nc.vector.copy_predicated(sel[0:118], qsm[0:118], F_all[0:118, 1])
            nc.vector.copy_predicated(sel[0:118], c1m[0:118], F_all[0:118, 0])
            nc.vector.copy_predicated(sel[0:118], c2m[0:118], F_all[0:118, 2])
            him = ev.tile([128, B, WIN], bf16, tag="him")
            hi = ev.tile([128, B, WIN], bf16, tag="hi", bufs=4)
            midm = ev.tile([128, B, WIN], bf16, tag="midm")
            mid = ev.tile([128, B, WIN], bf16, tag="mid", bufs=4)
            nc.vector.tensor_single_scalar(him[0:118], wsl(G), HIGH,
                                           AluOpType.is_gt)
            nc.gpsimd.tensor_tensor(hi[0:118], sel[0:118], him[0:118],
                                    AluOpType.mult)
            nc.vector.scalar_tensor_tensor(midm[0:118], wsl(G), LOW, him[0:118],
                                           AluOpType.is_ge, AluOpType.is_gt)
            nc.gpsimd.tensor_tensor(mid[0:118], midm[0:118], sel[0:118],
                                    AluOpType.mult)
            rs2 = ev.tile([128, B, RB], bf16, tag="rs2", bufs=4)
            nc.vector.tensor_tensor(rs2[0:118], hi[0:118, :, 0:RB],
                                    hi[0:118, :, 2:2 + RB], AluOpType.add)
            nc.vector.tensor_tensor(rs2[0:118], rs2[0:118],
                                    hi[0:118, :, 1:1 + RB], AluOpType.add)
            hyst[w] = (hi, mid, rs2)

        # ---- phase 3: column-shifted rowsums + final combine ----
        for w in range(NW):
            hi, mid, rs2 = hyst[w]
            rsp = ev.tile([128, B, RB], bf16, tag="rsp")
            rsm = ev.tile([128, B, RB], bf16, tag="rsm")
            nc.sync.dma_start(rsp[0:117], rs2[1:118])
            if w + 1 < NW:
                nc.sync.dma_start(rsp[117:118], hyst[w + 1][2][0:1])
            nc.sync.dma_start(rsm[1:118], rs2[0:117])
            if w > 0:
                nc.sync.dma_start(rsm[0:1], hyst[w - 1][2][117:118])
            s33 = ev.tile([128, B, RB], bf16, tag="s33")
            nc.gpsimd.tensor_tensor(s33[0:118], rsp[0:118], rsm[0:118],
                                    AluOpType.add)
            nc.gpsimd.tensor_tensor(s33[0:118], s33[0:118], rs2[0:118],
                                    AluOpType.add)
            cond = ev.tile([128, B, RB], bf16, tag="cond")
            om = ev.tile([128, B, RB], bf16, tag="om")
            outw = ev.tile([128, B, RB], f32, tag="outw")
            nc.vector.tensor_tensor(cond[0:118], s33[0:118],
                                    hi[0:118, :, 1:1 + RB], AluOpType.is_gt)
            nc.gpsimd.tensor_tensor(om[0:118], cond[0:118],
                                    mid[0:118, :, 1:1 + RB], AluOpType.mult)
            nc.vector.tensor_tensor(outw[0:118], om[0:118],
                                    hi[0:118, :, 1:1 + RB], AluOpType.max)
            nc.sync.dma_start(o_d[:, w], outw[0:118])
    return nc


def _prep_weights(gauss_h):
    g = np.asarray(gauss_h, np.float64).reshape(-1)
    wa = np.stack([_band(np.convolve(g, [1., 2., 1.]), XR, 68),
                   _band(np.convolve(g, [1., 0., -1.]), XR, 68)], axis=1)
    wb = np.stack([_band(np.convolve(g, [1., 0., -1.]), 128, 118),
                   _band(np.convolve(g, [1., 2., 1.]), 128, 118)], axis=1)
    return np.ascontiguousarray(wa, np.float32), np.ascontiguousarray(wb, np.float32)


def kernel(img, gauss_h, gauss_v, sobel_h, sobel_v, directional, connect):
    img = np.asarray(img, np.float32)
    wa, wb = _prep_weights(gauss_h)

    if "nc" not in _CACHE:
        nc = _build()
        _split_excess_waits(nc)
        _CACHE["nc"] = nc
    nc = _CACHE["nc"]

    xp = np.zeros((B, C, H + 14, W + 14), np.float32)
    xp[:, :, 7:7 + H, 7:7 + W] = img
    in_maps = []
    for c in range(NCORES):
        r0 = RB * c
        slab = np.ascontiguousarray(
            xp[:, :, r0:r0 + XR, :].reshape(B * C, XR, XC).transpose(1, 0, 2))
        in_maps.append({"x": slab, "wa": wa, "wb": wb})

    global LAST_EXEC_NS
    if TRACE:
        res = run_bass_kernel_spmd(nc, in_maps, core_ids=list(range(NCORES)),
                                   trace=True)
        LAST_EXEC_NS = res.exec_time_ns
    else:
        res = run_bass_kernel_spmd(nc, in_maps, core_ids=list(range(NCORES)))

    out = np.zeros((B, 1, H, W), np.float32)
    for c in range(NCORES):
        o = res.results[c]["o"]                      # [118, NW, B, RB]
        r0 = RB * c
        for w in range(NW):
            _, _, mw = _chunk_dims(w)
            p_lo = 2 if w == 0 else 0
            f_lo = CW * w + p_lo - 2
            f_hi = min(W, CW * w + mw - 2)
            n = f_hi - f_lo
            if n <= 0:
                continue
            out[:, 0, r0:r0 + RB, f_lo:f_hi] = np.transpose(
                o[p_lo:p_lo + n, w], (1, 2, 0))
    out[:, :, 0, :] = 0.0
    out[:, :, -1, :] = 0.0
    out[:, :, :, 0] = 0.0
    out[:, :, :, -1] = 0.0
    return out


def _split_excess_waits(nc, max_waits=1):
    """This walrus build allows one sync-wait per instruction; move excess
    waits onto preceding same-engine sequencer NoOps (queues are in-order)."""
    ctr = 0
    for f in nc.m.functions:
        for blk in f.blocks:
            out = []
            for inst in blk.instructions:
                si = inst.sync_info
                if si is not None and len(si.on_wait) > max_waits:
                    waits = list(si.on_wait)
                    excess, keep = waits[:-max_waits], waits[-max_waits:]
                    for i in range(0, len(excess), max_waits):
                        ctr += 1
                        nop = mybir.InstNoOp(name=f"waitfix-{ctr}", ins=[], outs=[])
                        nop.engine = inst.engine
                        nop.sync_info = mybir.SyncInfo(
                            on_wait=excess[i:i + max_waits], on_update=[])
                        out.append(nop)
                    inst.sync_info = mybir.SyncInfo(
                        on_wait=keep, on_update=list(si.on_update))
                out.append(inst)
            blk.instructions = out
    return ctr



# revision 19
# speedup vs baseline: 1.5755x; 1.5755x over previous
"""Trainium2 Bass kernel for nn_Canny_61100204753382 (8-core SPMD).

Sharding: spatial row-bands (64 output rows x all 8 images per core). The
reference's flat-gather quirk reads all_filtered[k_pos, b, i, j] - the
direction index lands in the batch slot and the pixel's own batch index
selects the direction offset - so the coupling between images is at the SAME
pixel position and row-band sharding stays core-local given a small row halo.

Pipeline per column-chunk (CW=117 output cols from 128 input cols, +1
extended G column so each chunk owns its right-neighbor boundary):
  stepA: vertical 11-tap banded matmuls in bf16 hi/lo 3-pass (error ~2^-17);
  stepB: horizontal banded matmuls in f32, 2 images per call; per-channel
  squares evacuated via ACT, magnitude summed on Pool; channel-summed
  gxs/gys (f32) for sector masks; NMS with 4 direct direction compares and
  4 derived via NOT(shifted C) (exact modulo float ties, masked by the
  thresholds); hysteresis via row/col sums; conv(w) -> nms(w) -> fin(w-1)
  interleave keeps PE/DVE/ACT/Pool all busy.
"""

import math
import numpy as np
from contextlib import ExitStack

import concourse.bass as bass
import concourse.mybir as mybir
import concourse.tile as tile
from concourse.bass_utils import run_bass_kernel_spmd
from concourse.alu_op_type import AluOpType

f32 = mybir.dt.float32
f32r = mybir.dt.float32r
bf16 = mybir.dt.bfloat16
u8 = mybir.dt.uint8
AF = mybir.ActivationFunctionType

B, C, H, W = 8, 3, 512, 512
NCORES = 8
RB = H // NCORES          # output rows per core
XR = RB + 14              # input rows per core (7-row halo each side)
XC = W + 14               # padded cols
GR = RB + 4               # G rows per band (final rows -2..65)
CW = 117                  # chunk stride (128 in-cols -> 117 out-cols + 1 ext)
NW = -(-(W + 4) // CW)    # 5 column chunks
PC = CW                   # partitions used by NMS ops
PE1 = CW + 1              # extended G columns per chunk
WIN = RB + 2              # is_max row window (final rows -1..64)
T1 = float(math.tan(math.pi / 8))
T2 = float(math.tan(3 * math.pi / 8))
LOW, HIGH = 0.1, 0.3
NEIGH = [(0, 1), (1, 1), (1, 0), (1, -1), (0, -1), (-1, -1), (-1, 0), (-1, 1)]

_CACHE = {}
TRACE = False
LAST_EXEC_NS = None


def _band(comp, K, M, taps=11):
    Wb = np.zeros((K, M), np.float32)
    for k in range(K):
        for m in range(M):
            if 0 <= k - m < taps:
                Wb[k, m] = comp[k - m]
    return Wb


def _chunk_dims(w):
    s = CW * w
    kw = min(128, XC - s)           # in-cols this chunk
    mw = min(CW, (W + 4) - s)       # out (G) cols this chunk
    return s, kw, mw


def _build():
    nc = bass.Bass()
    # x pre-chunked on host: [XR, NW, B*C, 128] (chunk w zero-padded to 128)
    xh_d = nc.dram_tensor("xh", [XR, NW, B * C, 128], bf16, kind="ExternalInput")
    xl_d = nc.dram_tensor("xl", [XR, NW, B * C, 128], bf16, kind="ExternalInput")
    wah_d = nc.dram_tensor("wah", [XR, 2, 68], bf16, kind="ExternalInput")
    wal_d = nc.dram_tensor("wal", [XR, 2, 68], bf16, kind="ExternalInput")
    wb_d = nc.dram_tensor("wb", [128, 2, PE1], f32, kind="ExternalInput")
    o_d = nc.dram_tensor("o", [PC, NW, B, RB], bf16, kind="ExternalOutput")

    with tile.TileContext(nc) as tc, ExitStack() as ctx:
        P = ctx.enter_context
        const = P(tc.tile_pool(name="const", bufs=1))
        big = P(tc.tile_pool(name="big", bufs=1))
        xp = P(tc.tile_pool(name="xp", bufs=2))
        ev = P(tc.tile_pool(name="ev", bufs=2))
        nm = P(tc.tile_pool(name="nm", bufs=1))
        psA = P(tc.tile_pool(name="psA", bufs=2, space="PSUM"))
        psB = P(tc.tile_pool(name="psB", bufs=2, space="PSUM"))
        psS = P(tc.tile_pool(name="psS", bufs=2, space="PSUM"))

        wah_sb = const.tile([XR, 2, 68], bf16, tag="wah_sb")
        nc.sync.dma_start(wah_sb[:], wah_d[:])
        wal_sb = const.tile([XR, 2, 68], bf16, tag="wal_sb")
        nc.sync.dma_start(wal_sb[:], wal_d[:])
        wb_sb = const.tile([128, 2, PE1], f32, tag="wb_sb")
        nc.sync.dma_start(wb_sb[:], wb_d[:])

        G = big.tile([128, NW, B, GR], f32, name="G", tag="G")
        GXS = big.tile([128, NW, B, GR], f32, tag="GXS")
        GYS = big.tile([128, NW, B, GR], f32, tag="GYS")
        Gp1 = big.tile([128, NW, B, GR], f32, tag="Gp1")
        Gm1 = big.tile([128, NW, B, GR], f32, tag="Gm1")
        c1m = big.tile([128, NW, B, WIN], u8, tag="c1m")
        c2m = big.tile([128, NW, B, WIN], u8, tag="c2m")
        qsm = big.tile([128, NW, B, WIN], u8, tag="qsm")
        F = big.tile([128, NW, 8, 4, WIN], bf16, tag="F")
        him = big.tile([128, NW, B, WIN], bf16, tag="him")
        hi = big.tile([128, NW, B, WIN], bf16, tag="hi")
        mid = big.tile([128, NW, B, WIN], bf16, tag="mid")
        rs2 = big.tile([128, NW, B, RB], bf16, tag="rs2")
        cball = big.tile([128, 8, B, GR], bf16, tag="cball")
        ftmp = big.tile([128, 4, 4, WIN], bf16, tag="ftmp")
        nc.vector.memset(cball[0:1, 3:6], 0.0)

        def conv_chunk(w):
            s, kw, mw = _chunk_dims(w)
            mwE = min(PE1, (W + 4) - s)
            xh_sb = xp.tile([XR, B * C, 128], bf16, tag="xh_sb")
            nc.sync.dma_start(xh_sb[:], xh_d[:, w])
            xl_sb = xp.tile([XR, B * C, 128], bf16, tag="xl_sb")
            nc.sync.dma_start(xl_sb[:], xl_d[:, w])
            gxA = ev.tile([128, B, C, 2, 68], f32, tag="gxA")
            for img in range(B):
                pa = psA.tile([128, 3, 2, 68], f32, tag="pa")
                for ci in range(3):
                    lhT = xh_sb[0:XR, img * C + ci, 0:kw]
                    llT = xl_sb[0:XR, img * C + ci, 0:kw]
                    nc.tensor.matmul(pa[0:kw, ci], lhT, wah_sb[0:XR],
                                     start=True, stop=False)
                    nc.tensor.matmul(pa[0:kw, ci], lhT, wal_sb[0:XR],
                                     start=False, stop=False)
                    nc.tensor.matmul(pa[0:kw, ci], llT, wah_sb[0:XR],
                                     start=False, stop=True)
                nc.scalar.copy(gxA[0:kw, img], pa[0:kw])
            gsA = ev.tile([128, B, 2, 68], f32, tag="gsA")
            sq = ev.tile([128, B, 2, 3, GR], f32, tag="sq", bufs=1)
            mag = ev.tile([128, B, 3, GR], f32, tag="mag", bufs=1)
            tg = ev.tile([128, B, GR], f32, tag="tg", bufs=1)
            for img in range(1, B, 2):
                pb = psB.tile([PE1, 2, 512], f32, tag="pb")
                for j in range(2):
                    nc.tensor.matmul(pb[0:mwE, j, 0:408], wb_sb[0:kw, j, 0:mwE],
                                     gxA[0:kw, img - 1:img + 1, :, j],
                                     start=True, stop=True)
                    nc.scalar.square(sq[0:mwE, img - 1:img + 1, j],
                                     pb[0:mwE, j, 0:408])
                if img % 4 == 3:
                    h = img // 4
                    hs = slice(h * 4, h * 4 + 4)
                    nc.gpsimd.tensor_tensor(gsA[:, hs], gxA[:, hs, 0],
                                            gxA[:, hs, 1], AluOpType.add)
                    nc.gpsimd.tensor_tensor(gsA[:, hs], gsA[:, hs],
                                            gxA[:, hs, 2], AluOpType.add)
                    for j in range(2):
                        pS = psS.tile([PC, 4, 68], f32, tag="pS")
                        nc.tensor.matmul(pS[0:mw], wb_sb[0:kw, j, 0:mw],
                                         gsA[0:kw, hs, j],
                                         start=True, stop=True)
                        dst = (GXS if j == 0 else GYS)
                        nc.scalar.copy(dst[0:PC, w, hs], pS[0:PC])
                    nc.gpsimd.tensor_tensor(mag[0:mwE, hs], sq[0:mwE, hs, 0],
                                            sq[0:mwE, hs, 1], AluOpType.add)
                    nc.scalar.sqrt(mag[0:mwE, hs], mag[0:mwE, hs])
                    nc.gpsimd.tensor_tensor(tg[0:mwE, hs], mag[0:mwE, hs, 0],
                                            mag[0:mwE, hs, 1], AluOpType.add)
                    nc.gpsimd.tensor_tensor(G[0:mwE, w, hs], tg[0:mwE, hs],
                                            mag[0:mwE, hs, 2], AluOpType.add)

        def nms_chunk(w):
            # sector masks (f32 exact)
            wax = GXS[0:PC, w, :, 1:1 + WIN]
            way = GYS[0:PC, w, :, 1:1 + WIN]
            qpr = ev.tile([128, B, WIN], bf16, tag="qpr")
            nc.vector.tensor_tensor(qpr[0:PC], wax, way, AluOpType.mult)
            nc.vector.tensor_single_scalar(qsm[0:PC, w], qpr[0:PC], 0.0,
                                           AluOpType.is_ge)
            nc.scalar.activation(GXS[0:PC, w], GXS[0:PC, w], AF.Abs)
            nc.scalar.activation(GYS[0:PC, w], GYS[0:PC, w], AF.Abs)
            nc.vector.scalar_tensor_tensor(c1m[0:PC, w], wax, T1, way,
                                           AluOpType.mult, AluOpType.is_gt)
            nc.vector.scalar_tensor_tensor(c2m[0:PC, w], wax, T2, way,
                                           AluOpType.mult, AluOpType.is_lt)
            # column-shifted G (self-contained: G has PE1 cols)
            nc.sync.dma_start(Gp1[0:PC, w], G[1:PE1, w])
            nc.sync.dma_start(Gm1[1:PC, w], G[0:PC - 1, w])
            if w > 0:
                nc.sync.dma_start(Gm1[0:1, w], G[PC - 1:PC, w - 1])
            Gw = G[0:PC, w, :, 1:1 + WIN]
            # derived slots 3,4,5,6 = shifted copies of the direct planes;
            # boundary partition 0 from the PREVIOUS chunk (emitted first so
            # it reads the old planes)
            if w > 0:
                nc.sync.dma_start(cball[0:1, 4, :, 1:67],
                                  cball[PC - 1:PC, 0, :, 1:67])
                nc.sync.dma_start(cball[0:1, 5, :, 1:67],
                                  cball[PC - 1:PC, 1, :, 0:66])
                nc.sync.dma_start(cball[0:1, 3, :, 1:67],
                                  cball[PC - 1:PC, 7, :, 2:68])
            # direct compares b = 0,1,2,7; partners derived as NOT(shifted C)
            # (exact modulo float ties, which the threshold masks zero out)
            nc.vector.tensor_tensor(cball[0:PC, 0], G[0:PC, w],
                                    Gp1[0:PC, w], AluOpType.is_gt)
            nc.vector.tensor_tensor(cball[0:PC, 1, :, 0:67],
                                    G[0:PC, w, :, 0:67],
                                    Gp1[0:PC, w, :, 1:68], AluOpType.is_gt)
            nc.vector.tensor_tensor(cball[0:PC, 2, :, 0:67],
                                    G[0:PC, w, :, 0:67],
                                    G[0:PC, w, :, 1:68], AluOpType.is_gt)
            nc.vector.tensor_tensor(cball[0:PC, 7, :, 1:68],
                                    G[0:PC, w, :, 1:68],
                                    Gp1[0:PC, w, :, 0:67], AluOpType.is_gt)
            nc.sync.dma_start(cball[1:PC, 4, :, 1:67],
                              cball[0:PC - 1, 0, :, 1:67])
            nc.sync.dma_start(cball[1:PC, 5, :, 1:67],
                              cball[0:PC - 1, 1, :, 0:66])
            nc.sync.dma_start(cball[1:PC, 3, :, 1:67],
                              cball[0:PC - 1, 7, :, 2:68])
            nc.sync.dma_start(cball[0:PC, 6, :, 1:67],
                              cball[0:PC, 2, :, 0:66])
            # F: direct pairs multiply; derived pairs NOT a AND NOT b
            nc.vector.tensor_tensor(F[0:PC, w, 0:3],
                                    cball[0:PC, 0:3, 0:4, 1:67],
                                    cball[0:PC, 0:3, 4:8, 1:67],
                                    AluOpType.mult)
            nc.vector.tensor_tensor(F[0:PC, w, 7:8],
                                    cball[0:PC, 7:8, 0:4, 1:67],
                                    cball[0:PC, 7:8, 4:8, 1:67],
                                    AluOpType.mult)
            nc.vector.tensor_tensor(ftmp[0:PC],
                                    cball[0:PC, 3:7, 0:4, 1:67],
                                    cball[0:PC, 3:7, 4:8, 1:67],
                                    AluOpType.add)
            nc.vector.tensor_single_scalar(F[0:PC, w, 3:7], ftmp[0:PC],
                                           0.5, AluOpType.is_lt)
            # 4-way select by sector class
            sel = ev.tile([128, B, WIN], bf16, tag="sel", bufs=1)
            nc.vector.tensor_copy(sel[0:PC], F[0:PC, w, :, 3])
            nc.vector.copy_predicated(sel[0:PC], qsm[0:PC, w], F[0:PC, w, :, 1])
            nc.vector.copy_predicated(sel[0:PC], c1m[0:PC, w], F[0:PC, w, :, 0])
            nc.vector.copy_predicated(sel[0:PC], c2m[0:PC, w], F[0:PC, w, :, 2])
            # hysteresis masks
            nc.vector.tensor_single_scalar(him[0:PC, w], Gw, HIGH,
                                           AluOpType.is_gt)
            nc.vector.scalar_tensor_tensor(mid[0:PC, w], Gw, LOW,
                                           him[0:PC, w],
                                           AluOpType.is_ge, AluOpType.is_gt)
            nc.vector.tensor_tensor(mid[0:PC, w], mid[0:PC, w], sel[0:PC],
                                    AluOpType.mult)
            nc.vector.tensor_tensor(hi[0:PC, w], sel[0:PC], him[0:PC, w],
                                    AluOpType.mult)
            nc.vector.tensor_copy(rs2[0:PC, w], hi[0:PC, w, :, 1:1 + RB])
            nc.vector.tensor_tensor(rs2[0:PC, w], rs2[0:PC, w],
                                    hi[0:PC, w, :, 0:RB], AluOpType.add)
            nc.vector.tensor_tensor(rs2[0:PC, w], rs2[0:PC, w],
                                    hi[0:PC, w, :, 2:2 + RB], AluOpType.add)

        rsp = nm.tile([128, NW, B, RB], bf16, tag="rsp")
        rsm = nm.tile([128, NW, B, RB], bf16, tag="rsm")

        def fin_chunk(w):
            nc.sync.dma_start(rsp[0:PC - 1, w], rs2[1:PC, w])
            if w + 1 < NW:
                nc.sync.dma_start(rsp[PC - 1:PC, w], rs2[0:1, w + 1])
            nc.sync.dma_start(rsm[1:PC, w], rs2[0:PC - 1, w])
            if w > 0:
                nc.sync.dma_start(rsm[0:1, w], rs2[PC - 1:PC, w - 1])
            nc.vector.tensor_tensor(rsp[0:PC, w], rsp[0:PC, w],
                                    rsm[0:PC, w], AluOpType.add)
            nc.vector.tensor_tensor(rsp[0:PC, w], rsp[0:PC, w],
                                    rs2[0:PC, w], AluOpType.add)
            nc.vector.tensor_tensor(rsp[0:PC, w], rsp[0:PC, w],
                                    hi[0:PC, w, :, 1:1 + RB], AluOpType.is_gt)
            nc.vector.tensor_tensor(rsp[0:PC, w], rsp[0:PC, w],
                                    mid[0:PC, w, :, 1:1 + RB], AluOpType.mult)
            nc.vector.tensor_tensor(rsp[0:PC, w], rsp[0:PC, w],
                                    hi[0:PC, w, :, 1:1 + RB], AluOpType.max)
            nc.sync.dma_start(o_d[:, w], rsp[0:PC, w])

        # interleave: conv(w), nms(w), fin(w-1)
        for w in range(NW):
            conv_chunk(w)
            nms_chunk(w)
            if w >= 1:
                fin_chunk(w - 1)
        fin_chunk(NW - 1)
    return nc


def _prep_weights(gauss_h):
    g = np.asarray(gauss_h, np.float64).reshape(-1)
    wa = np.stack([_band(np.convolve(g, [1., 2., 1.]), XR, 68),
                   _band(np.convolve(g, [1., 0., -1.]), XR, 68)], axis=1)
    wb = np.stack([_band(np.convolve(g, [1., 0., -1.]), 128, PE1),
                   _band(np.convolve(g, [1., 2., 1.]), 128, PE1)], axis=1)
    return np.ascontiguousarray(wa, np.float32), np.ascontiguousarray(wb, np.float32)


def kernel(img, gauss_h, gauss_v, sobel_h, sobel_v, directional, connect):
    import ml_dtypes
    bf = ml_dtypes.bfloat16
    img = np.asarray(img, np.float32)
    wa, wb = _prep_weights(gauss_h)
    wa_hi = wa.astype(bf)
    wa_lo = (wa - wa_hi.astype(np.float32)).astype(bf)

    if "nc" not in _CACHE:
        nc = _build()
        _split_excess_waits(nc)
        _CACHE["nc"] = nc
    nc = _CACHE["nc"]

    xp = np.zeros((B, C, H + 14, NW * CW + 11), np.float32)
    xp[:, :, 7:7 + H, 7:7 + W] = img
    in_maps = []
    for c in range(NCORES):
        r0 = RB * c
        slab = xp[:, :, r0:r0 + XR, :].reshape(B * C, XR, -1)
        chunks = np.stack([slab[:, :, CW * w:CW * w + 128] for w in range(NW)],
                          axis=0)                       # [NW, B*C, XR, 128]
        xin = np.ascontiguousarray(chunks.transpose(2, 0, 1, 3))
        x_hi = xin.astype(bf)
        x_lo = (xin - x_hi.astype(np.float32)).astype(bf)
        in_maps.append({"xh": x_hi, "xl": x_lo, "wah": wa_hi, "wal": wa_lo,
                        "wb": wb})

    global LAST_EXEC_NS
    if TRACE:
        res = run_bass_kernel_spmd(nc, in_maps, core_ids=list(range(NCORES)),
                                   trace=True)
        LAST_EXEC_NS = res.exec_time_ns
    else:
        res = run_bass_kernel_spmd(nc, in_maps, core_ids=list(range(NCORES)))

    out = np.zeros((B, 1, H, W), np.float32)
    for c in range(NCORES):
        o = np.asarray(res.results[c]["o"], np.float32)  # [PC, NW, B, RB]
        r0 = RB * c
        for w in range(NW):
            _, _, mw = _chunk_dims(w)
            p_lo = 2 if w == 0 else 0
            f_lo = CW * w + p_lo - 2
            f_hi = min(W, CW * w + mw - 2)
            n = f_hi - f_lo
            if n <= 0:
                continue
            out[:, 0, r0:r0 + RB, f_lo:f_hi] = np.transpose(
                o[p_lo:p_lo + n, w], (1, 2, 0))
    out[:, :, 0, :] = 0.0
    out[:, :, -1, :] = 0.0
    out[:, :, :, 0] = 0.0
    out[:, :, :, -1] = 0.0
    return out


def _split_excess_waits(nc, max_waits=1):
    """This walrus build allows one sync-wait per instruction; move excess
    waits onto preceding same-engine sequencer NoOps (queues are in-order)."""
    ctr = 0
    for f in nc.m.functions:
        for blk in f.blocks:
            out = []
            for inst in blk.instructions:
                si = inst.sync_info
                if si is not None and len(si.on_wait) > max_waits:
                    waits = list(si.on_wait)
                    excess, keep = waits[:-max_waits], waits[-max_waits:]
                    for i in range(0, len(excess), max_waits):
                        ctr += 1
                        nop = mybir.InstNoOp(name=f"waitfix-{ctr}", ins=[], outs=[])
                        nop.engine = inst.engine
                        nop.sync_info = mybir.SyncInfo(
                            on_wait=excess[i:i + max_waits], on_update=[])
                        out.append(nop)
                    inst.sync_info = mybir.SyncInfo(
                        on_wait=keep, on_update=list(si.on_update))
                    out.append(inst)
                else:
                    out.append(inst)
            blk.instructions = out
    return ctr


# revision 21
# speedup vs baseline: 1.5766x; 1.0007x over previous
"""Trainium2 Bass kernel for nn_Canny_61100204753382 (8-core SPMD).

Sharding: spatial row-bands (64 output rows x all 8 images per core). The
reference's flat-gather quirk reads all_filtered[k_pos, b, i, j] - the
direction index lands in the batch slot and the pixel's own batch index
selects the direction offset - so the coupling between images is at the SAME
pixel position and row-band sharding stays core-local given a small row halo.

Pipeline per column-chunk (CW=117 output cols from 128 input cols, +1
extended G column so each chunk owns its right-neighbor boundary):
  stepA: vertical 11-tap banded matmuls in bf16 hi/lo 3-pass (error ~2^-17);
  stepB: horizontal banded matmuls in f32, 2 images per call; per-channel
  squares evacuated via ACT, magnitude summed on Pool; channel-summed
  gxs/gys (f32) for sector masks; NMS with 4 direct direction compares and
  4 derived via NOT(shifted C) (exact modulo float ties, masked by the
  thresholds); hysteresis via row/col sums; conv(w) -> nms(w) -> fin(w-1)
  interleave keeps PE/DVE/ACT/Pool all busy.
"""

import math
import numpy as np
from contextlib import ExitStack

import concourse.bass as bass
import concourse.mybir as mybir
import concourse.tile as tile
from concourse.bass_utils import run_bass_kernel_spmd
from concourse.alu_op_type import AluOpType

f32 = mybir.dt.float32
f32r = mybir.dt.float32r
bf16 = mybir.dt.bfloat16
u8 = mybir.dt.uint8
AF = mybir.ActivationFunctionType

B, C, H, W = 8, 3, 512, 512
NCORES = 8
RB = H // NCORES          # output rows per core
XR = RB + 14              # input rows per core (7-row halo each side)
XC = W + 14               # padded cols
GR = RB + 4               # G rows per band (final rows -2..65)
CW = 117                  # chunk stride (128 in-cols -> 117 out-cols + 1 ext)
NW = -(-(W + 4) // CW)    # 5 column chunks
PC = CW                   # partitions used by NMS ops
PE1 = CW + 1              # extended G columns per chunk
WIN = RB + 2              # is_max row window (final rows -1..64)
T1 = float(math.tan(math.pi / 8))
T2 = float(math.tan(3 * math.pi / 8))
LOW, HIGH = 0.1, 0.3
NEIGH = [(0, 1), (1, 1), (1, 0), (1, -1), (0, -1), (-1, -1), (-1, 0), (-1, 1)]

_CACHE = {}
TRACE = False
LAST_EXEC_NS = None


def _band(comp, K, M, taps=11):
    Wb = np.zeros((K, M), np.float32)
    for k in range(K):
        for m in range(M):
            if 0 <= k - m < taps:
                Wb[k, m] = comp[k - m]
    return Wb


def _chunk_dims(w):
    s = CW * w
    kw = min(128, XC - s)           # in-cols this chunk
    mw = min(CW, (W + 4) - s)       # out (G) cols this chunk
    return s, kw, mw


def _build():
    nc = bass.Bass()
    # x pre-chunked on host: [XR, NW, B*C, 128] (chunk w zero-padded to 128)
    xh_d = nc.dram_tensor("xh", [XR, NW, B * C, 128], bf16, kind="ExternalInput")
    xl_d = nc.dram_tensor("xl", [XR, NW, B * C, 128], bf16, kind="ExternalInput")
    wah_d = nc.dram_tensor("wah", [XR, 2, 68], bf16, kind="ExternalInput")
    wal_d = nc.dram_tensor("wal", [XR, 2, 68], bf16, kind="ExternalInput")
    wb_d = nc.dram_tensor("wb", [128, 2, PE1], f32, kind="ExternalInput")
    o_d = nc.dram_tensor("o", [PC, NW, B, RB], bf16, kind="ExternalOutput")

    with tile.TileContext(nc) as tc, ExitStack() as ctx:
        P = ctx.enter_context
        const = P(tc.tile_pool(name="const", bufs=1))
        big = P(tc.tile_pool(name="big", bufs=1))
        xp = P(tc.tile_pool(name="xp", bufs=2))
        ev = P(tc.tile_pool(name="ev", bufs=2))
        nm = P(tc.tile_pool(name="nm", bufs=1))
        psA = P(tc.tile_pool(name="psA", bufs=2, space="PSUM"))
        psB = P(tc.tile_pool(name="psB", bufs=2, space="PSUM"))
        psS = P(tc.tile_pool(name="psS", bufs=2, space="PSUM"))

        wah_sb = const.tile([XR, 2, 68], bf16, tag="wah_sb")
        nc.sync.dma_start(wah_sb[:], wah_d[:])
        wal_sb = const.tile([XR, 2, 68], bf16, tag="wal_sb")
        nc.sync.dma_start(wal_sb[:], wal_d[:])
        wb_sb = const.tile([128, 2, PE1], f32, tag="wb_sb")
        nc.sync.dma_start(wb_sb[:], wb_d[:])

        G = big.tile([128, NW, B, GR], f32, name="G", tag="G")
        GXS = big.tile([128, NW, B, GR], f32, tag="GXS")
        GYS = big.tile([128, NW, B, GR], f32, tag="GYS")
        Gp1 = big.tile([128, NW, B, GR], f32, tag="Gp1")
        c1m = big.tile([128, NW, B, WIN], u8, tag="c1m")
        c2m = big.tile([128, NW, B, WIN], u8, tag="c2m")
        qsm = big.tile([128, NW, B, WIN], u8, tag="qsm")
        F = big.tile([128, NW, 8, 4, WIN], bf16, tag="F")
        him = big.tile([128, NW, B, WIN], bf16, tag="him")
        hi = big.tile([128, NW, B, WIN], bf16, tag="hi")
        mid = big.tile([128, NW, B, WIN], bf16, tag="mid")
        rs2 = big.tile([128, NW, B, RB], bf16, tag="rs2")
        rsc = big.tile([128, NW, B, RB], bf16, tag="rsc")
        cball = big.tile([128, 8, B, GR], bf16, tag="cball")
        ftmp = big.tile([128, 4, 4, WIN], bf16, tag="ftmp")
        nc.vector.memset(cball[0:1, 3:6], 0.0)

        def conv_chunk(w):
            s, kw, mw = _chunk_dims(w)
            mwE = min(PE1, (W + 4) - s)
            xh_sb = xp.tile([XR, B * C, 128], bf16, tag="xh_sb")
            nc.sync.dma_start(xh_sb[:], xh_d[:, w])
            xl_sb = xp.tile([XR, B * C, 128], bf16, tag="xl_sb")
            nc.sync.dma_start(xl_sb[:], xl_d[:, w])
            gxA = ev.tile([128, B, C, 2, 68], f32, tag="gxA")
            for img in range(B):
                pa = psA.tile([128, 3, 2, 68], f32, tag="pa")
                for ci in range(3):
                    lhT = xh_sb[0:XR, img * C + ci, 0:kw]
                    llT = xl_sb[0:XR, img * C + ci, 0:kw]
                    nc.tensor.matmul(pa[0:kw, ci], lhT, wah_sb[0:XR],
                                     start=True, stop=False)
                    nc.tensor.matmul(pa[0:kw, ci], lhT, wal_sb[0:XR],
                                     start=False, stop=False)
                    nc.tensor.matmul(pa[0:kw, ci], llT, wah_sb[0:XR],
                                     start=False, stop=True)
                nc.scalar.copy(gxA[0:kw, img], pa[0:kw])
            gsA = ev.tile([128, B, 2, 68], f32, tag="gsA")
            sq = ev.tile([128, B, 2, 3, GR], f32, tag="sq", bufs=1)
            mag = ev.tile([128, B, 3, GR], f32, tag="mag", bufs=1)
            tg = ev.tile([128, B, GR], f32, tag="tg", bufs=1)
            for img in range(1, B, 2):
                pb = psB.tile([PE1, 2, 512], f32, tag="pb")
                for j in range(2):
                    nc.tensor.matmul(pb[0:mwE, j, 0:408], wb_sb[0:kw, j, 0:mwE],
                                     gxA[0:kw, img - 1:img + 1, :, j],
                                     start=True, stop=True)
                    nc.scalar.square(sq[0:mwE, img - 1:img + 1, j],
                                     pb[0:mwE, j, 0:408])
                if img % 4 == 3:
                    h = img // 4
                    hs = slice(h * 4, h * 4 + 4)
                    nc.gpsimd.tensor_tensor(gsA[:, hs], gxA[:, hs, 0],
                                            gxA[:, hs, 1], AluOpType.add)
                    nc.gpsimd.tensor_tensor(gsA[:, hs], gsA[:, hs],
                                            gxA[:, hs, 2], AluOpType.add)
                    for j in range(2):
                        pS = psS.tile([PC, 4, 68], f32, tag="pS")
                        nc.tensor.matmul(pS[0:mw], wb_sb[0:kw, j, 0:mw],
                                         gsA[0:kw, hs, j],
                                         start=True, stop=True)
                        dst = (GXS if j == 0 else GYS)
                        nc.scalar.copy(dst[0:PC, w, hs], pS[0:PC])
                    nc.gpsimd.tensor_tensor(mag[0:mwE, hs], sq[0:mwE, hs, 0],
                                            sq[0:mwE, hs, 1], AluOpType.add)
                    nc.scalar.sqrt(mag[0:mwE, hs], mag[0:mwE, hs])
                    nc.gpsimd.tensor_tensor(tg[0:mwE, hs], mag[0:mwE, hs, 0],
                                            mag[0:mwE, hs, 1], AluOpType.add)
                    nc.gpsimd.tensor_tensor(G[0:mwE, w, hs], tg[0:mwE, hs],
                                            mag[0:mwE, hs, 2], AluOpType.add)

        def nms_chunk(w):
            # sector masks (f32 exact)
            wax = GXS[0:PC, w, :, 1:1 + WIN]
            way = GYS[0:PC, w, :, 1:1 + WIN]
            qpr = ev.tile([128, B, WIN], bf16, tag="qpr")
            nc.vector.tensor_tensor(qpr[0:PC], wax, way, AluOpType.mult)
            nc.vector.tensor_single_scalar(qsm[0:PC, w], qpr[0:PC], 0.0,
                                           AluOpType.is_ge)
            nc.scalar.activation(GXS[0:PC, w], GXS[0:PC, w], AF.Abs)
            nc.scalar.activation(GYS[0:PC, w], GYS[0:PC, w], AF.Abs)
            nc.vector.scalar_tensor_tensor(c1m[0:PC, w], wax, T1, way,
                                           AluOpType.mult, AluOpType.is_gt)
            nc.vector.scalar_tensor_tensor(c2m[0:PC, w], wax, T2, way,
                                           AluOpType.mult, AluOpType.is_lt)
            # column-shifted G (self-contained: G has PE1 cols)
            nc.sync.dma_start(Gp1[0:PC, w], G[1:PE1, w])
            Gw = G[0:PC, w, :, 1:1 + WIN]
            # derived slots 3,4,5,6 = shifted copies of the direct planes;
            # boundary partition 0 from the PREVIOUS chunk (emitted first so
            # it reads the old planes)
            if w > 0:
                nc.sync.dma_start(cball[0:1, 4, :, 1:67],
                                  cball[PC - 1:PC, 0, :, 1:67])
                nc.sync.dma_start(cball[0:1, 5, :, 1:67],
                                  cball[PC - 1:PC, 1, :, 0:66])
                nc.sync.dma_start(cball[0:1, 3, :, 1:67],
                                  cball[PC - 1:PC, 7, :, 2:68])
            # direct compares b = 0,1,2,7; partners derived as NOT(shifted C)
            # (exact modulo float ties, which the threshold masks zero out)
            nc.vector.tensor_tensor(cball[0:PC, 0], G[0:PC, w],
                                    Gp1[0:PC, w], AluOpType.is_gt)
            nc.vector.tensor_tensor(cball[0:PC, 1, :, 0:67],
                                    G[0:PC, w, :, 0:67],
                                    Gp1[0:PC, w, :, 1:68], AluOpType.is_gt)
            nc.vector.tensor_tensor(cball[0:PC, 2, :, 0:67],
                                    G[0:PC, w, :, 0:67],
                                    G[0:PC, w, :, 1:68], AluOpType.is_gt)
            nc.vector.tensor_tensor(cball[0:PC, 7, :, 1:68],
                                    G[0:PC, w, :, 1:68],
                                    Gp1[0:PC, w, :, 0:67], AluOpType.is_gt)
            nc.sync.dma_start(cball[1:PC, 4, :, 1:67],
                              cball[0:PC - 1, 0, :, 1:67])
            nc.sync.dma_start(cball[1:PC, 5, :, 1:67],
                              cball[0:PC - 1, 1, :, 0:66])
            nc.sync.dma_start(cball[1:PC, 3, :, 1:67],
                              cball[0:PC - 1, 7, :, 2:68])
            nc.sync.dma_start(cball[0:PC, 6, :, 1:67],
                              cball[0:PC, 2, :, 0:66])
            # F: direct pairs multiply; derived pairs NOT a AND NOT b
            nc.vector.tensor_tensor(F[0:PC, w, 0:3],
                                    cball[0:PC, 0:3, 0:4, 1:67],
                                    cball[0:PC, 0:3, 4:8, 1:67],
                                    AluOpType.mult)
            nc.vector.tensor_tensor(F[0:PC, w, 7:8],
                                    cball[0:PC, 7:8, 0:4, 1:67],
                                    cball[0:PC, 7:8, 4:8, 1:67],
                                    AluOpType.mult)
            nc.vector.tensor_tensor(ftmp[0:PC],
                                    cball[0:PC, 3:7, 0:4, 1:67],
                                    cball[0:PC, 3:7, 4:8, 1:67],
                                    AluOpType.add)
            nc.vector.tensor_single_scalar(F[0:PC, w, 3:7], ftmp[0:PC],
                                           0.5, AluOpType.is_lt)
            # 4-way select by sector class
            sel = ev.tile([128, B, WIN], bf16, tag="sel", bufs=1)
            nc.vector.tensor_copy(sel[0:PC], F[0:PC, w, :, 3])
            nc.vector.copy_predicated(sel[0:PC], qsm[0:PC, w], F[0:PC, w, :, 1])
            nc.vector.copy_predicated(sel[0:PC], c1m[0:PC, w], F[0:PC, w, :, 0])
            nc.vector.copy_predicated(sel[0:PC], c2m[0:PC, w], F[0:PC, w, :, 2])
            # hysteresis masks
            nc.vector.tensor_single_scalar(him[0:PC, w], Gw, HIGH,
                                           AluOpType.is_gt)
            nc.vector.scalar_tensor_tensor(mid[0:PC, w], Gw, LOW,
                                           him[0:PC, w],
                                           AluOpType.is_ge, AluOpType.is_gt)
            nc.vector.tensor_tensor(mid[0:PC, w], mid[0:PC, w], sel[0:PC],
                                    AluOpType.mult)
            nc.vector.tensor_tensor(hi[0:PC, w], sel[0:PC], him[0:PC, w],
                                    AluOpType.mult)
            nc.vector.tensor_tensor(rsc[0:PC, w], hi[0:PC, w, :, 0:RB],
                                    hi[0:PC, w, :, 2:2 + RB], AluOpType.add)
            nc.vector.tensor_tensor(rs2[0:PC, w], rsc[0:PC, w],
                                    hi[0:PC, w, :, 1:1 + RB], AluOpType.add)

        rsp = nm.tile([128, NW, B, RB], bf16, tag="rsp")
        rsm = nm.tile([128, NW, B, RB], bf16, tag="rsm")

        def fin_chunk(w):
            nc.sync.dma_start(rsp[0:PC - 1, w], rs2[1:PC, w])
            if w + 1 < NW:
                nc.sync.dma_start(rsp[PC - 1:PC, w], rs2[0:1, w + 1])
            nc.sync.dma_start(rsm[1:PC, w], rs2[0:PC - 1, w])
            if w > 0:
                nc.sync.dma_start(rsm[0:1, w], rs2[PC - 1:PC, w - 1])
            # s8 = 8-neighbor sum of hi; om = cond*mid = min(mid, s8);
            # out = max(om, hi)
            nc.vector.tensor_tensor(rsp[0:PC, w], rsp[0:PC, w],
                                    rsm[0:PC, w], AluOpType.add)
            nc.vector.tensor_tensor(rsp[0:PC, w], rsp[0:PC, w],
                                    rsc[0:PC, w], AluOpType.add)
            nc.vector.tensor_tensor(rsp[0:PC, w], rsp[0:PC, w],
                                    mid[0:PC, w, :, 1:1 + RB], AluOpType.min)
            nc.vector.tensor_tensor(rsp[0:PC, w], rsp[0:PC, w],
                                    hi[0:PC, w, :, 1:1 + RB], AluOpType.max)
            nc.sync.dma_start(o_d[:, w], rsp[0:PC, w])

        # interleave: conv(w), nms(w), fin(w-1)
        for w in range(NW):
            conv_chunk(w)
            nms_chunk(w)
            if w >= 1:
                fin_chunk(w - 1)
        fin_chunk(NW - 1)
    return nc


def _prep_weights(gauss_h):
    g = np.asarray(gauss_h, np.float64).reshape(-1)
    wa = np.stack([_band(np.convolve(g, [1., 2., 1.]), XR, 68),
                   _band(np.convolve(g, [1., 0., -1.]), XR, 68)], axis=1)
    wb = np.stack([_band(np.convolve(g, [1., 0., -1.]), 128, PE1),
                   _band(np.convolve(g, [1., 2., 1.]), 128, PE1)], axis=1)
    return np.ascontiguousarray(wa, np.float32), np.ascontiguousarray(wb, np.float32)


def kernel(img, gauss_h, gauss_v, sobel_h, sobel_v, directional, connect):
    import ml_dtypes
    bf = ml_dtypes.bfloat16
    img = np.asarray(img, np.float32)
    wa, wb = _prep_weights(gauss_h)
    wa_hi = wa.astype(bf)
    wa_lo = (wa - wa_hi.astype(np.float32)).astype(bf)

    if "nc" not in _CACHE:
        nc = _build()
        _split_excess_waits(nc)
        _CACHE["nc"] = nc
    nc = _CACHE["nc"]

    xp = np.zeros((B, C, H + 14, NW * CW + 11), np.float32)
    xp[:, :, 7:7 + H, 7:7 + W] = img
    in_maps = []
    for c in range(NCORES):
        r0 = RB * c
        slab = xp[:, :, r0:r0 + XR, :].reshape(B * C, XR, -1)
        chunks = np.stack([slab[:, :, CW * w:CW * w + 128] for w in range(NW)],
                          axis=0)                       # [NW, B*C, XR, 128]
        xin = np.ascontiguousarray(chunks.transpose(2, 0, 1, 3))
        x_hi = xin.astype(bf)
        x_lo = (xin - x_hi.astype(np.float32)).astype(bf)
        in_maps.append({"xh": x_hi, "xl": x_lo, "wah": wa_hi, "wal": wa_lo,
                        "wb": wb})

    global LAST_EXEC_NS
    if TRACE:
        res = run_bass_kernel_spmd(nc, in_maps, core_ids=list(range(NCORES)),
                                   trace=True)
        LAST_EXEC_NS = res.exec_time_ns
    else:
        res = run_bass_kernel_spmd(nc, in_maps, core_ids=list(range(NCORES)))

    out = np.zeros((B, 1, H, W), np.float32)
    for c in range(NCORES):
        o = np.asarray(res.results[c]["o"], np.float32)  # [PC, NW, B, RB]
        r0 = RB * c
        for w in range(NW):
            _, _, mw = _chunk_dims(w)
            p_lo = 2 if w == 0 else 0
            f_lo = CW * w + p_lo - 2
            f_hi = min(W, CW * w + mw - 2)
            n = f_hi - f_lo
            if n <= 0:
                continue
            out[:, 0, r0:r0 + RB, f_lo:f_hi] = np.transpose(
                o[p_lo:p_lo + n, w], (1, 2, 0))
    out[:, :, 0, :] = 0.0
    out[:, :, -1, :] = 0.0
    out[:, :, :, 0] = 0.0
    out[:, :, :, -1] = 0.0
    return out


def _split_excess_waits(nc, max_waits=1):
    """This walrus build allows one sync-wait per instruction; move excess
    waits onto preceding same-engine sequencer NoOps (queues are in-order)."""
    ctr = 0
    for f in nc.m.functions:
        for blk in f.blocks:
            out = []
            for inst in blk.instructions:
                si = inst.sync_info
                if si is not None and len(si.on_wait) > max_waits:
                    waits = list(si.on_wait)
                    excess, keep = waits[:-max_waits], waits[-max_waits:]
                    for i in range(0, len(excess), max_waits):
                        ctr += 1
                        nop = mybir.InstNoOp(name=f"waitfix-{ctr}", ins=[], outs=[])
                        nop.engine = inst.engine
                        nop.sync_info = mybir.SyncInfo(
                            on_wait=excess[i:i + max_waits], on_update=[])
                        out.append(nop)
                    inst.sync_info = mybir.SyncInfo(
                        on_wait=keep, on_update=list(si.on_update))
                    out.append(inst)
                else:
                    out.append(inst)
            blk.instructions = out
    return ctr
